# revision 1
# baseline (speedup 1.0000x reference)
"""CoupledFourierSystem Trainium2 kernel.

Math: out[t,e] = sum_d W[e,d] * sum_{h,c} A[d,h,c]*cos(w[d,h,c]*s[t]+phi[d,h,c]) + b[e]

Flatten j=(d,h,c) -> 2048.  With G[j,e] = A_j * W[e,d(j)]:
    out[t,e] = sum_j cos(w_j*s_t + phi_j) * G[j,e] + b[e]
cos(x) = sin(x + pi/2);  sin(theta) computed via turns:
    u = (w_j/2pi)*s_t + p2_j      (p2 = (phi+pi/2)/2pi + 4, keeps u > 0)
    frac = u mod 1; arg = frac - 0.5        -> sin(2pi*arg) = -sin(theta)
so G is negated on the host.  Per core (t-shard of 4096):
    DVE:  u = s_b*f[p] + p2[p]          (tensor_scalar, 2 elem/cyc fp32 SBUF)
    DVE/GpSimd (alternating): arg = (u mod 1) - 0.5
    ACT:  sin(2pi*arg)                  (bottleneck: 16 ops of [128,4096])
    PE :  psum[c] += G_jt.T @ sin[:,c]  (16x8 fp32 matmuls, K=128,M=64,N=512)
Output [64, 4096] DMA'd straight from PSUM; host concatenates, transposes,
adds b.
"""
import numpy as np
from contextlib import ExitStack

import concourse.bass as bass
import concourse.tile as tile
import concourse.dve_ops as dve_ops
from concourse import mybir
from concourse.bass_utils import run_bass_kernel_spmd
from concourse.dve_table_gen import dve_ver_for
from concourse.vector_clock import ScopedClock, VectorClock

S, DIM, H = 32768, 64, 16
NCORES = 8
T = S // NCORES          # 4096 time points per core
J = DIM * H * 2          # 2048 flattened harmonics
NJT = J // 128           # 16 j-tiles
NCH = T // 512           # 8 psum chunks
f32 = mybir.dt.float32
TWO_PI = 2.0 * np.pi


# --- workaround: walrus rejects the TileContext exit drain when it carries
# >2 sem waits ("Too many sync wait commands").  Split the waits onto
# preceding SP nops (one wait each); SP is in-order so the drain still runs
# only after every outstanding proc completed.
def _split_drain_and_barrier(self, tick_clock, wait_clock):
    gc = tick_clock.global_clock
    ticks = eval(repr(gc).replace("VectorClock", ""))
    nprocs = len(ticks)
    for i, t in enumerate(ticks):
        if t == 0:
            continue
        sub = [0] * nprocs
        sub[i] = t
        nop = self.nc.sync.nop(nofuse=True, hint=f"drain_wait_p{i}")
        wait_clock.add_sem_waits(nop.ins, ScopedClock({None: VectorClock(sub)}))
    self.nc.sync.drain()
    self.nc.all_engine_barrier()
    assert self.sems is not None
    popped = self.nc._tile_sem_poison_stack.pop()
    assert popped is self._sem_poison
    self.nc.clear_and_free_semaphores(list(self.sems.allocated().values()))
    self.nc.all_engine_barrier()


tile.TileContext._drain_and_barrier = _split_drain_and_barrier

MAX_WAITS = 1
GP_JTS = 9          # j-tiles whose passes 1-2 run on GpSimd
BCAST_MODE = "dma"  # s-broadcast path: "dma" (verified) or "mm" (experimental)
MAGIC = 1.5 * 2.0 ** 23     # forces RNE-to-integer for |u| < 2^22




def _split_excess_waits(nc: bass.Bass):
    """Walrus rejects instructions carrying more than a couple of sem waits.
    Hoist excess waits onto preceding same-engine nops (engines are in-order,
    so semantics are unchanged)."""
    import copy
    m = nc.m
    new_module = copy.replace(m, functions=[])
    nid = [0]
    for function in m.functions:
        new_function = copy.replace(function, blocks=[])
        new_function.set_allocations_from_list(function.allocations)
        for block in m.functions[0].blocks if False else function.blocks:
            new_insts = []
            for inst in block.instructions:
                si = inst.sync_info
                if si is not None and len(si.on_wait) > MAX_WAITS:
                    waits = list(si.on_wait)
                    extra, keep = waits[:-MAX_WAITS], waits[-MAX_WAITS:]
                    for w_i in range(0, len(extra), MAX_WAITS):
                        nid[0] += 1
                        nop = mybir.InstNoOp(
                            name=f"{inst.name}-wsplit{nid[0]}",
                            sync_info=mybir.SyncInfo(
                                on_wait=extra[w_i:w_i + MAX_WAITS], on_update=[]
                            ),
                            bass_nofuse=True,
                            engine=inst.engine,
                        )
                        new_insts.append(nop)
                    inst.sync_info = mybir.SyncInfo(
                        on_wait=keep, on_update=list(si.on_update)
                    )
                new_insts.append(inst)
            new_block = copy.replace(block, instructions=new_insts)
            new_function.blocks.append(new_block)
        new_module.functions.append(new_function)
    nc.m = new_module


def build_nc(reps: int = 1, split_waits: bool = True) -> bass.Bass:
    nc = bass.Bass()
    s_d = nc.declare_dram_parameter("s", [T], f32, isOutput=False)
    f_d = nc.declare_dram_parameter("fv", [128, NJT], f32, isOutput=False)
    p_d = nc.declare_dram_parameter("pv", [128, NJT], f32, isOutput=False)
    g_d = nc.declare_dram_parameter("g", [128, NJT, 64], f32, isOutput=False)
    o_d = nc.declare_dram_parameter("out", [64, T], f32, isOutput=True)

    with tile.TileContext(nc) as tc, ExitStack() as ctx:
        const = ctx.enter_context(tc.tile_pool(name="const", bufs=1))
        upool = ctx.enter_context(tc.tile_pool(name="upool", bufs=2))
        kpool = ctx.enter_context(tc.tile_pool(name="kpool", bufs=2))
        apool = ctx.enter_context(tc.tile_pool(name="apool", bufs=2))
        spool = ctx.enter_context(tc.tile_pool(name="spool", bufs=3))
        psum = ctx.enter_context(tc.tile_pool(name="psum", bufs=1, space="PSUM"))

        fv_sb = const.tile([128, NJT], f32)
        pv_sb = const.tile([128, NJT], f32)
        g_sb = const.tile([128, NJT, 64], f32)
        nc.sync.dma_start(out=fv_sb, in_=f_d[:, :])
        nc.sync.dma_start(out=pv_sb, in_=p_d[:, :])
        nc.sync.dma_start(out=g_sb, in_=g_d[:, :, :])
        s_b = const.tile([128, T], f32)

        if BCAST_MODE == "mm":
            ones_sb = const.tile([1, 128], f32)
            s_row = const.tile([1, T], f32)
            nc.vector.memset(ones_sb, 1.0)
            nc.sync.dma_start(out=s_row, in_=s_d[:].reshape(1, T))

        for _ in range(reps):
            if BCAST_MODE == "mm":
                # broadcast via K=1 matmul: psum[p, t] = ones[1,p].T @ s[1,t]
                for c in range(NCH):
                    bps = psum.tile([128, 512], f32, tag=f"bc{c % 2}",
                                    name=f"bc{c}")
                    nc.tensor.matmul(
                        bps, ones_sb, s_row[:, c * 512:(c + 1) * 512],
                        start=True, stop=True,
                    )
                    nc.vector.tensor_copy(s_b[:, c * 512:(c + 1) * 512], bps)
            else:
                # broadcast s across partitions, one DMA per 512-chunk
                for c in range(NCH):
                    sl = s_d[c * 512:(c + 1) * 512]
                    bcast = bass.AP(
                        tensor=sl.tensor, offset=sl.offset,
                        ap=[[0, 128]] + [list(x) for x in sl.ap],
                    )
                    nc.sync.dma_start(out=s_b[:, c * 512:(c + 1) * 512], in_=bcast)

            psums = [
                psum.tile([64, 512], f32, tag=f"ps{c}", name=f"ps{c}")
                for c in range(NCH)
            ]
            for jt in range(NJT):
                # u = s*f + p2 ; k = rne(u) via +/- magic ; arg = u - k
                eng = nc.gpsimd if jt < GP_JTS else nc.vector
                u_t = upool.tile([128, T], f32, tag="u", name=f"u{jt}")
                eng.tensor_scalar(
                    u_t, s_b, fv_sb[:, jt:jt + 1], pv_sb[:, jt:jt + 1],
                    mybir.AluOpType.mult, mybir.AluOpType.add,
                )
                k_t = kpool.tile([128, T], f32, tag="k", name=f"k{jt}")
                eng.tensor_scalar(
                    k_t, u_t, MAGIC, MAGIC,
                    mybir.AluOpType.add, mybir.AluOpType.subtract,
                )
                a_t = apool.tile([128, T], f32, tag="a", name=f"a{jt}")
                nc.vector.tensor_tensor(
                    a_t, u_t, k_t, mybir.AluOpType.subtract
                )
                sin_t = spool.tile([128, T], f32, tag="sin", name=f"sin{jt}")
                nc.scalar.activation(
                    sin_t, a_t, mybir.ActivationFunctionType.Sin,
                    bias=0.0, scale=TWO_PI,
                )
                for c in range(NCH):
                    nc.tensor.matmul(
                        psums[c], g_sb[:, jt, :], sin_t[:, c * 512:(c + 1) * 512],
                        start=(jt == 0), stop=(jt == NJT - 1),
                    )
            for c in range(NCH):
                o_sb = spool.tile([64, 512], f32, tag="o", name=f"o{c}", bufs=4)
                nc.vector.tensor_copy(o_sb, psums[c])
                nc.sync.dma_start(
                    out=o_d[:, c * 512:(c + 1) * 512], in_=o_sb
                )
    if split_waits:
        _split_excess_waits(nc)
    return nc


def _prep_in_maps(s, A, phi, w, W):
    w_flat = np.asarray(w, np.float64).reshape(J)
    phi_flat = np.asarray(phi, np.float64).reshape(J)
    A_flat = np.asarray(A, np.float64).reshape(J)
    d_of_j = np.arange(J) // (H * 2)

    fv = (w_flat / TWO_PI).astype(np.float32).reshape(NJT, 128).T.copy()
    pv = ((phi_flat + np.pi / 2) / TWO_PI).astype(np.float32) \
        .reshape(NJT, 128).T.copy()
    G = (A_flat[:, None] * np.asarray(W, np.float64).T[d_of_j, :])
    g = G.astype(np.float32).reshape(NJT, 128, 64).transpose(1, 0, 2).copy()

    s_np = np.asarray(s, np.float32)
    return [
        {"s": s_np[i * T:(i + 1) * T].copy(), "fv": fv, "pv": pv, "g": g}
        for i in range(NCORES)
    ]


def kernel(s, x, A, phi, w, W, b):
    in_maps = _prep_in_maps(s, A, phi, w, W)
    nc = build_nc(reps=1)
    res = run_bass_kernel_spmd(nc, in_maps, core_ids=list(range(NCORES)))
    parts = [res.results[i]["out"] for i in range(NCORES)]      # each [64, T]
    full = np.concatenate(parts, axis=1).T                      # [S, 64]
    return (full + np.asarray(b, np.float32)[None, :]).astype(np.float32)



# revision 37
# speedup vs baseline: 293.8537x; 293.8537x over previous
"""CoupledFourierSystem Trainium2 kernel — Fourier-extension basis version.

Math: out[t,e] = sum_d W[e,d] * sum_{h,c} A[d,h,c]*cos(w[d,h,c]*s[t]+phi[d,h,c]) + b[e]

All 2048 harmonics j=(d,h,c) have |w_j| <= 20 rad, s in [0,1).  Host-side
PARAMETER folding (depends only on A/phi/w/W, not on the input s):
approximate every cos(w_j s + phi_j) in one shared band-limited basis
    psi_k(s) = sin(2*pi*(fv_k s + pv_k)),  k = 0..NB-1
(Fourier-extension basis, period L_EXT > 1, least-squares fit on [0,1];
residual ~2e-4), then fold the per-harmonic coefficients into the linear
layer:  out[t,e] ~= sum_k psi_k(s_t) R[k,e] + b[e].
Device transcendental work drops from S*J to S*NB sins (70x) and the
matmul contraction from K=2048 to K=29.

Device work per core (t-shard of T=4096, stacked NBLK=4 time blocks x 32
partitions; TB=1024 free dim processed in NCH=2 chunks of C=512):
    PE  : u_psum[p,i] = fv_p*s[blk(p)*TB+i] + pv_p as ONE K=18 bf16 matmul
          per chunk (lhsT = block-masked bf16 splits of fv/pv, rhs = bf16
          splits of s; bf16xbf16 products are exact in fp32, so phases are
          good to ~9e-5 turns).  Beats a DMA partition-broadcast of s,
          which is packet-bound at ~83ns per partition-line.
    DVE : k = (u+MAGIC)-MAGIC (RNE round); a = u - k in [-.5,.5]
    ACT : psi = Sin(2*pi*a) -> fp16
    PE  : out_psum[2-bank pair] = R.T @ psi per block (fp16, K=32; block 3
          uses a zero-padded K=64 lhsT since partition 96 is an illegal
          matmul base)
    DVE/ACT: 2-bank psum -> sbuf fp16 (one wide drain per pair), DMA out
          on alternating queues
Host: concat cores, transpose, + b (fp32).  ~22.5us/launch incl ~11us of
NRT preamble/postamble; measured rel err 3.2e-4 (gate 2e-2).
"""
import numpy as np
from contextlib import ExitStack

import concourse.bass as bass
import concourse.tile as tile
from concourse import mybir
from concourse.bass_utils import run_bass_kernel_spmd
from concourse.vector_clock import ScopedClock, VectorClock

S, DIM, H = 32768, 64, 16
NCORES = 8
T = S // NCORES          # 4096 time points per core
NBLK = 4                 # time blocks stacked on the partition axis
PB = 128 // NBLK         # partitions per block (32)
TB = T // NBLK           # 1024 time points per block
NCH = 2                  # free-dim chunks of the stacked tile
C = TB // NCH            # 512
J = DIM * H * 2          # 2048 flattened harmonics
f32 = mybir.dt.float32
f16 = mybir.dt.float16
bf16 = mybir.dt.bfloat16
TWO_PI = 2.0 * np.pi
MAGIC = 1.5 * 2.0 ** 23     # forces RNE-to-integer for |u| < 2^22
# phase matmul: u = fv*s + pv in bf16 splits (fv = f1+f2+f3, s = s1+s2+s3,
# bf16 x bf16 products are exact in fp32); keep the 4 largest cross terms
# (residual ~ fv*2^-18 turns, well under the error budget).
PAIRS = [(0, 0), (0, 1), (1, 0), (1, 1)]
KPH = NBLK * len(PAIRS) + 2  # 18 lhsT rows for the phase matmul

# basis: sin(2*pi*(fv s + pv)) with fv = k/L (k=0..KMAX cos rows then
# k=1..KMAX sin rows), least-squares fit on [0,1], L_EXT>1 extension
L_EXT = 1.25
KMAX = 14
NB = 2 * KMAX + 1        # 29 basis functions, padded to PB=32 partitions
NFIT = 1025


# --- workaround: walrus rejects the TileContext exit drain when it carries
# >2 sem waits ("Too many sync wait commands").  Split the waits onto
# preceding SP nops (one wait each); SP is in-order so the drain still runs
# only after every outstanding proc completed.
def _split_drain_and_barrier(self, tick_clock, wait_clock):
    gc = tick_clock.global_clock
    ticks = eval(repr(gc).replace("VectorClock", ""))
    nprocs = len(ticks)
    for i, t in enumerate(ticks):
        if t == 0:
            continue
        sub = [0] * nprocs
        sub[i] = t
        nop = self.nc.sync.nop(nofuse=True, hint=f"drain_wait_p{i}")
        wait_clock.add_sem_waits(nop.ins, ScopedClock({None: VectorClock(sub)}))
    self.nc.sync.drain()
    self.nc.all_engine_barrier()
    assert self.sems is not None
    popped = self.nc._tile_sem_poison_stack.pop()
    assert popped is self._sem_poison
    self.nc.clear_and_free_semaphores(list(self.sems.allocated().values()))
    self.nc.all_engine_barrier()


tile.TileContext._drain_and_barrier = _split_drain_and_barrier

MAX_WAITS = 1


def _split_excess_waits(nc: bass.Bass):
    """Walrus rejects instructions carrying more than a couple of sem waits.
    Hoist excess waits onto preceding same-engine nops (engines are in-order,
    so semantics are unchanged)."""
    import copy
    m = nc.m
    new_module = copy.replace(m, functions=[])
    nid = [0]
    for function in m.functions:
        new_function = copy.replace(function, blocks=[])
        new_function.set_allocations_from_list(function.allocations)
        for block in function.blocks:
            new_insts = []
            for inst in block.instructions:
                si = inst.sync_info
                if si is not None and len(si.on_wait) > MAX_WAITS:
                    waits = list(si.on_wait)
                    extra, keep = waits[:-MAX_WAITS], waits[-MAX_WAITS:]
                    for w_i in range(0, len(extra), MAX_WAITS):
                        nid[0] += 1
                        nop = mybir.InstNoOp(
                            name=f"{inst.name}-wsplit{nid[0]}",
                            sync_info=mybir.SyncInfo(
                                on_wait=extra[w_i:w_i + MAX_WAITS], on_update=[]
                            ),
                            bass_nofuse=True,
                            engine=inst.engine,
                        )
                        new_insts.append(nop)
                    inst.sync_info = mybir.SyncInfo(
                        on_wait=keep, on_update=list(si.on_update)
                    )
                new_insts.append(inst)
            new_block = copy.replace(block, instructions=new_insts)
            new_function.blocks.append(new_block)
        new_module.functions.append(new_function)
    nc.m = new_module


def build_nc(reps: int = 1, split_waits: bool = True) -> bass.Bass:
    nc = bass.Bass()
    # Phase generation via one K=KPH bf16 matmul per chunk (PE broadcast —
    # avoids the per-partition-line packetization cost of a DMA broadcast):
    #   psum_u[p, i] = fv_p * s[blk(p)*TB + i] + pv_p
    # computed exactly from bf16 splits; lhsT f-rows are masked per block.
    s8_d = nc.declare_dram_parameter("s8", [KPH, TB], bf16, isOutput=False)
    fp_d = nc.declare_dram_parameter("fp", [KPH, 128], bf16, isOutput=False)
    r_d = nc.declare_dram_parameter("r", [128, 64], f16, isOutput=False)
    r3_d = nc.declare_dram_parameter("r3", [128, 64], f16, isOutput=False)
    o_d = nc.declare_dram_parameter("out", [64, T], f16, isOutput=True)

    with tile.TileContext(nc) as tc, ExitStack() as ctx:
        const = ctx.enter_context(tc.tile_pool(name="const", bufs=1))
        work = ctx.enter_context(tc.tile_pool(name="work", bufs=2))
        psum = ctx.enter_context(tc.tile_pool(name="psum", bufs=1, space="PSUM"))

        s8_sb = const.tile([KPH, TB], bf16)
        fp_sb = const.tile([KPH, 128], bf16)
        r_sb = const.tile([128, 64], f16)
        r3_sb = const.tile([128, 64], f16)
        nc.gpsimd.dma_start(out=fp_sb, in_=fp_d[:, :])
        nc.sync.dma_start(out=s8_sb, in_=s8_d[:, :])
        nc.gpsimd.dma_start(out=r_sb, in_=r_d[:, :])
        nc.gpsimd.dma_start(out=r3_sb, in_=r3_d[:, :])

        o_ap = o_d[:, :]
        out_engines = [nc.sync, nc.gpsimd, nc.sync, nc.gpsimd]
        for _ in range(reps):
            for ch in range(NCH):
                u_ps = psum.tile(
                    [128, C], f32, tag=f"u{ch}", name=f"u{ch}"
                )
                nc.tensor.matmul(
                    u_ps, fp_sb, s8_sb[:, ch * C:(ch + 1) * C],
                    start=True, stop=True,
                )
                k_t = work.tile([128, C], f32, tag=f"k{ch}", name=f"k{ch}")
                nc.vector.tensor_scalar(
                    k_t, u_ps, MAGIC, MAGIC,
                    mybir.AluOpType.add, mybir.AluOpType.subtract,
                )
                a_t = work.tile([128, C], f32, tag=f"a{ch}", name=f"a{ch}")
                nc.vector.tensor_tensor(
                    a_t, u_ps, k_t, mybir.AluOpType.subtract
                )
                psi_t = work.tile([128, C], f16, tag=f"p{ch}", name=f"psi{ch}")
                nc.scalar.activation(
                    psi_t, a_t, mybir.ActivationFunctionType.Sin,
                    bias=0.0, scale=TWO_PI,
                )
                # block pairs share a 2-bank psum tile -> one wide drain
                for pair in range(2):
                    # 3 distinct 2-bank tags fit PSUM next to u0/u1; the
                    # last (ch1, pair1) reuses (ch0, pair0)'s banks
                    ps = psum.tile(
                        [64, 2 * C], f32, tag=f"pp{(ch * 2 + pair) % 3}",
                        name=f"pp{ch}{pair}",
                    )
                    for sub in range(2):
                        blk = pair * 2 + sub
                        if blk < 3:
                            # bases 0/32/64 are legal matmul start partitions
                            nc.tensor.matmul(
                                ps[:, sub * C:(sub + 1) * C],
                                r_sb[blk * PB:(blk + 1) * PB, :],
                                psi_t[blk * PB:(blk + 1) * PB, :],
                                start=True, stop=True,
                            )
                        else:
                            # base 96 is illegal: K=64 matmul at base 64 with
                            # rows 64-95 of r3 zeroed so block 2 contributes 0
                            nc.tensor.matmul(
                                ps[:, sub * C:(sub + 1) * C],
                                r3_sb[64:128, :],
                                psi_t[64:128, :],
                                start=True, stop=True,
                            )
                    o_sb = work.tile(
                        [64, 2 * C], f16, tag=f"o{ch}{pair}",
                        name=f"o{ch}{pair}",
                    )
                    if pair == 0:
                        nc.vector.tensor_copy(o_sb, ps)
                    else:
                        nc.scalar.copy(o_sb, ps)
                    # out: DRAM [64, T]; dims (rows, 2 blocks, C)
                    out_ap = bass.AP(
                        tensor=o_ap.tensor,
                        offset=o_ap.offset + ch * C + pair * 2 * TB,
                        ap=[[T, 64], [TB, 2], [1, C]],
                    )
                    out_engines[ch * 2 + pair].dma_start(
                        out=out_ap, in_=o_sb
                    )
    if split_waits:
        _split_excess_waits(nc)
    return nc


def _fit_basis(A, phi, w, W):
    """Least-squares fit of all cos(w_j s + phi_j) in the shared basis;
    returns fv, pv [128,1] fp32 and R [128,64] fp16 (stacked NBLK times)."""
    ks = np.arange(KMAX + 1)
    fv = np.concatenate([ks / L_EXT, ks[1:] / L_EXT])          # turns/unit-s
    pv = np.concatenate([np.full(KMAX + 1, 0.25), np.zeros(KMAX)])

    s_dense = np.linspace(0.0, 1.0, NFIT)
    Phi = np.sin(TWO_PI * (s_dense[:, None] * fv[None, :] + pv[None, :]))
    U, sv, Vt = np.linalg.svd(Phi, full_matrices=False)
    keep = sv > 1e-7 * sv[0]
    Pinv = (Vt[keep].T / sv[keep]) @ U[:, keep].T               # [NB, NFIT]

    w_flat = np.asarray(w, np.float64).reshape(J)
    phi_flat = np.asarray(phi, np.float64).reshape(J)
    A_flat = np.asarray(A, np.float64).reshape(J)
    d_of_j = np.arange(J) // (H * 2)
    G = A_flat[:, None] * np.asarray(W, np.float64).T[d_of_j, :]   # [J, 64]

    F = np.cos(s_dense[:, None] * w_flat[None, :] + phi_flat[None, :])
    R = Pinv @ (F @ G)                                          # [NB, 64]

    def bf16_splits(x, n=3):
        """x (fp64) -> n bf16 arrays summing to x (residual splitting)."""
        import ml_dtypes
        outs, resid = [], np.asarray(x, np.float64)
        for _ in range(n):
            p = resid.astype(ml_dtypes.bfloat16)
            outs.append(p)
            resid = resid - p.astype(np.float64)
        return outs

    fsp = bf16_splits(fv)
    psp = bf16_splits(pv, n=2)
    import ml_dtypes
    fp26 = np.zeros((KPH, 128), ml_dtypes.bfloat16)
    r128 = np.zeros((128, 64), np.float16)
    r3 = np.zeros((128, 64), np.float16)
    for blk in range(NBLK):
        for t, (i, _) in enumerate(PAIRS):
            fp26[blk * len(PAIRS) + t, blk * PB: blk * PB + NB] = fsp[i]
        r128[blk * PB: blk * PB + NB, :] = R.astype(np.float16)
    for q in range(2):
        fp26[NBLK * len(PAIRS) + q, :] = np.concatenate(
            [np.pad(psp[q], (0, PB - NB)) for _ in range(NBLK)]
        )
    r3[3 * PB: 3 * PB + NB, :] = R.astype(np.float16)
    return fp26, r128, r3


def _prep_in_maps(s, A, phi, w, W):
    import ml_dtypes
    fp26, r128, r3 = _fit_basis(A, phi, w, W)
    s_np = np.asarray(s, np.float64)
    maps = []
    for i in range(NCORES):
        si = s_np[i * T:(i + 1) * T]
        s8 = np.ones((KPH, TB), ml_dtypes.bfloat16)
        for blk in range(NBLK):
            sb = si[blk * TB:(blk + 1) * TB]
            s1 = sb.astype(ml_dtypes.bfloat16)
            s2 = (sb - s1.astype(np.float64)).astype(ml_dtypes.bfloat16)
            s3 = (sb - s1.astype(np.float64) - s2.astype(np.float64)
                  ).astype(ml_dtypes.bfloat16)
            ssp = [s1, s2, s3]
            for t, (_, j) in enumerate(PAIRS):
                s8[blk * len(PAIRS) + t] = ssp[j]
        maps.append({"s8": s8, "fp": fp26, "r": r128, "r3": r3})
    return maps


def kernel(s, x, A, phi, w, W, b):
    in_maps = _prep_in_maps(s, A, phi, w, W)
    nc = build_nc(reps=1)
    res = run_bass_kernel_spmd(nc, in_maps, core_ids=list(range(NCORES)))
    parts = [res.results[i]["out"] for i in range(NCORES)]      # each [64, T] f16
    full = np.concatenate(parts, axis=1).T.astype(np.float32)   # [S, 64]
    return (full + np.asarray(b, np.float32)[None, :]).astype(np.float32)
